# revision 12
# baseline (speedup 1.0000x reference)
"""Trainium2 Bass kernel for nn_ExtendedAnomalyNet (patch-CNN over 24x24 map).

Algorithm: instead of running base_net on 576 overlapping 33x33 patches
(28.5 GMAC), we use the multiPool decomposition: conv1 is shared on the
padded image, and the two stride-2 maxpools are turned into 4/16
parity-indexed pooled maps, so conv2/conv3 are computed once per parity
combination (~1.1 GMAC total, 25x fewer FLOPs).

Sharding (8 cores): core c = (oy, ox, h): pool-parity (oy, ox) in {0,1}^2
and spatial half h in {0,1} (output rows i<12 vs i>=12). Every stage after
the host-built conv1 im2col is core-local; each core produces 72 of the
576 output pixels (512 features each). No collectives; the host gathers.

Matmul operands are fp16 (PSUM accumulation is fp32): same mantissa class
as the PE's fp32r mode but half the DMA bytes and full-rate weight loads.
Bias+LeakyReLU fused into one ACT Lrelu op per conv. Input DMAs issue in
parallel on separate engine queues; conv1's im2col streams in 4 chunks so
compute starts as soon as the first chunk lands.
"""
import numpy as np

IMH = IMW = 24

_CACHE = {}


def _host_prep(x, c1w, c1b, c2w, c2b, c3w, c3b, c4w, c4b, c5w, c5b, dw, db):
    xp = np.pad(np.asarray(x, np.float32)[0], ((0, 0), (16, 16), (16, 16)))  # (3,56,56)
    sw = np.lib.stride_tricks.sliding_window_view(xp, (5, 5), axis=(1, 2))  # (3,52,52,5,5)
    r1s = []
    for c in range(8):
        oy, ox, h = (c >> 2) & 1, (c >> 1) & 1, c & 1
        r0, c0 = oy + 12 * h, ox
        r1 = np.zeros((128, 38 * 50), np.float16)
        r1[:75] = (
            sw[:, r0:r0 + 38, c0:c0 + 50, :, :]
            .transpose(0, 3, 4, 1, 2)
            .reshape(75, 38 * 50)
        )
        r1s.append(r1)
    w1 = np.zeros((128, 128), np.float16)
    w1[:75] = np.asarray(c1w, np.float32).reshape(128, 75).T
    w2 = np.ascontiguousarray(
        np.asarray(c2w, np.float32).transpose(2, 3, 1, 0)  # (dy,dx,i,o)
    ).transpose(2, 0, 1, 3).reshape(128, 25 * 128).astype(np.float16)
    w3 = np.ascontiguousarray(
        np.asarray(c3w, np.float32).transpose(2, 3, 1, 0)
    ).transpose(2, 0, 1, 3).reshape(128, 25 * 128).astype(np.float16)
    w45d = np.zeros((128, 8, 128), np.float16)
    c4 = np.asarray(c4w, np.float32)[:, :, 0, 0]
    c5 = np.asarray(c5w, np.float32)[:, :, 0, 0]
    dwf = np.asarray(dw, np.float32)
    w45d[:, 0, :] = c4[:128, :].T
    w45d[:, 1, :] = c4[128:, :].T
    w45d[:, 2, :] = c5[:, :128].T
    w45d[:, 3, :] = c5[:, 128:].T
    for q in range(4):
        w45d[:, 4 + q, :] = dwf[128 * q:128 * (q + 1), :].T
    biases = np.zeros((128, 10), np.float32)
    biases[:, 0] = np.asarray(c1b, np.float32)
    biases[:, 1] = np.asarray(c2b, np.float32)
    biases[:, 2] = np.asarray(c3b, np.float32)
    biases[:, 3] = np.asarray(c4b, np.float32)[:128]
    biases[:, 4] = np.asarray(c4b, np.float32)[128:]
    biases[:, 5] = np.asarray(c5b, np.float32)
    biases[:, 6:10] = np.asarray(db, np.float32).reshape(4, 128).T
    return r1s, w1, w2, w3, w45d.reshape(128, 1024), biases


def _build_nc():
    from contextlib import ExitStack

    import concourse.bass as bass
    import concourse.bacc as bacc
    import concourse.mybir as mybir
    import concourse.tile as tile

    dt = mybir.dt
    AF = mybir.ActivationFunctionType

    nc = bacc.Bacc("TRN2", debug=False, num_devices=8)
    R1 = nc.dram_tensor("r1", [128, 1900], dt.float16, kind="ExternalInput").ap()
    W1 = nc.dram_tensor("w1", [128, 128], dt.float16, kind="ExternalInput").ap()
    W2 = nc.dram_tensor("w2", [128, 3200], dt.float16, kind="ExternalInput").ap()
    W3 = nc.dram_tensor("w3", [128, 3200], dt.float16, kind="ExternalInput").ap()
    W45 = nc.dram_tensor("w45d", [128, 1024], dt.float16, kind="ExternalInput").ap()
    BIAS = nc.dram_tensor("biases", [128, 10], dt.float32, kind="ExternalInput").ap()
    FEATS = nc.dram_tensor("feats", [128, 288], dt.float32, kind="ExternalOutput").ap()

    with tile.TileContext(nc) as tc, ExitStack() as ctx:
        const = ctx.enter_context(tc.tile_pool(name="const", bufs=1))
        work = ctx.enter_context(tc.tile_pool(name="work", bufs=1))
        ps = ctx.enter_context(tc.tile_pool(name="ps", bufs=4, space="PSUM"))

        # --- loads: parallel queues; r1 in 4 chunks so conv1 starts early ---
        r1c = [const.tile([128, 475], dt.float16, name=f"r1c{n}", tag=f"r1c{n}")
               for n in range(4)]
        w1t = const.tile([128, 128], dt.float16)
        bt = const.tile([128, 10], dt.float32)
        w2t = const.tile([128, 25, 128], dt.float16)
        w3t = const.tile([128, 25, 128], dt.float16)
        w45t = const.tile([128, 8, 128], dt.float16)
        for n in range(4):
            nc.sync.dma_start(out=r1c[n][:], in_=R1[:, 475 * n:475 * (n + 1)])
        nc.sync.dma_start(out=w1t[:], in_=W1)
        nc.sync.dma_start(out=bt[:], in_=BIAS)
        nc.scalar.dma_start(out=w2t[:], in_=W2.rearrange("p (t o) -> p t o", t=25))
        nc.gpsimd.dma_start(out=w3t[:], in_=W3.rearrange("p (t o) -> p t o", t=25))
        nc.sync.dma_start(out=w45t[:], in_=W45.rearrange("p (u o) -> p u o", u=8))

        scratch = work.tile([1, 2], dt.float32)
        nc.vector.memset(scratch[:], 0.0)
        nc.scalar.activation(out=scratch[:], in_=scratch[:], func=AF.Lrelu,
                             bias=0.0, scale=1.0, alpha=0.01)
        nc.scalar.activation(out=scratch[:], in_=scratch[:], func=AF.Identity,
                             bias=0.0, scale=1.0)

        def lrelu_bias(dst, src, bias_col):
            # dst = LeakyReLU(src + bias, slope 0.01) in one ACT op
            nc.scalar.activation(
                out=dst, in_=src, func=AF.Lrelu,
                bias=bt[:, bias_col:bias_col + 1], scale=1.0, alpha=0.01,
            )

        # --- conv1: 4 chunks of N=475 ---
        c1 = work.tile([128, 38, 50], dt.float16)
        c1f = c1[:].rearrange("p a b -> p (a b)")
        for n in range(4):
            pc = ps.tile([128, 475], dt.float32, tag="ps")
            nc.tensor.matmul(pc[:], w1t[:], r1c[n][:], start=True, stop=True)
            lrelu_bias(c1f[:, 475 * n:475 * (n + 1)], pc[:], 0)

        # --- pool1 -> P1 (128,19,25) fp16, per conv1 chunk (rows pipeline) ---
        c1v = c1[:].rearrange("p (u a) (v b) -> p u a v b", a=2, b=2)
        pa = work.tile([128, 19, 25], dt.float16)
        pb = work.tile([128, 19, 25], dt.float16)
        P1 = work.tile([128, 19, 25], dt.float16)
        for n in range(4):
            r0, r1_ = 5 * n, 5 * n + (5 if n < 3 else 4)
            nc.vector.tensor_max(out=pa[:, r0:r1_], in0=c1v[:, r0:r1_, 0, :, 0],
                                 in1=c1v[:, r0:r1_, 0, :, 1])
            nc.vector.tensor_max(out=pb[:, r0:r1_], in0=c1v[:, r0:r1_, 1, :, 0],
                                 in1=c1v[:, r0:r1_, 1, :, 1])
            nc.vector.tensor_max(out=P1[:, r0:r1_], in0=pa[:, r0:r1_],
                                 in1=pb[:, r0:r1_])

        # --- conv2: 25 accumulating matmuls, N=15x21=315 ---
        p2 = ps.tile([128, 15, 21], dt.float32, tag="ps")
        for dy in range(5):
            for dx in range(5):
                t = dy * 5 + dx
                nc.tensor.matmul(p2[:], w2t[:, t, :], P1[:, dy:dy + 15, dx:dx + 21],
                                 start=(t == 0), stop=(t == 24))
        c2 = work.tile([128, 15, 21], dt.float16)
        lrelu_bias(c2[:], p2[:], 1)

        # --- pool2 -> P2 (128, 4, 7, 10) fp16 (combos (py,px)) ---
        P2 = work.tile([128, 4, 7, 10], dt.float16)
        for py in range(2):
            for px in range(2):
                qa = work.tile([128, 7, 10], dt.float16, tag="p2a")
                qb = work.tile([128, 7, 10], dt.float16, tag="p2b")
                nc.vector.tensor_max(
                    out=qa[:],
                    in0=c2[:, py:py + 14:2, px:px + 19:2],
                    in1=c2[:, py + 1:py + 14:2, px:px + 19:2])
                nc.vector.tensor_max(
                    out=qb[:],
                    in0=c2[:, py:py + 14:2, px + 1:px + 20:2],
                    in1=c2[:, py + 1:py + 14:2, px + 1:px + 20:2])
                nc.vector.tensor_max(out=P2[:, py * 2 + px], in0=qa[:], in1=qb[:])

        # --- conv3: 25 accumulating matmuls, N=72 (combo, sl, t) ---
        p3 = ps.tile([128, 72], dt.float32, tag="ps")
        for e in range(5):
            for f in range(5):
                t = e * 5 + f
                nc.tensor.matmul(p3[:], w3t[:, t, :], P2[:, :, e:e + 3, f:f + 6],
                                 start=(t == 0), stop=(t == 24))
        h3 = work.tile([128, 72], dt.float16)
        lrelu_bias(h3[:], p3[:], 2)

        # --- conv4 (2 output halves) ---
        h4 = work.tile([128, 2, 72], dt.float16)
        for half in range(2):
            p4 = ps.tile([128, 72], dt.float32, tag="ps")
            nc.tensor.matmul(p4[:], w45t[:, half, :], h3[:], start=True, stop=True)
            lrelu_bias(h4[:, half], p4[:], 3 + half)

        # --- conv5 (accumulate 2 K-halves) ---
        p5 = ps.tile([128, 72], dt.float32, tag="ps")
        nc.tensor.matmul(p5[:], w45t[:, 2, :], h4[:, 0], start=True, stop=False)
        nc.tensor.matmul(p5[:], w45t[:, 3, :], h4[:, 1], start=False, stop=True)
        h5 = work.tile([128, 72], dt.float16)
        lrelu_bias(h5[:], p5[:], 5)

        # --- dense (4 output quarters), bias only, no activation ---
        out_t = work.tile([128, 4, 72], dt.float32)
        for q in range(4):
            pd = ps.tile([128, 72], dt.float32, tag="ps")
            nc.tensor.matmul(pd[:], w45t[:, 4 + q, :], h5[:], start=True, stop=True)
            nc.scalar.activation(out=out_t[:, q], in_=pd[:], func=AF.Identity,
                                 bias=bt[:, 6 + q:7 + q], scale=1.0)
        nc.sync.dma_start(out=FEATS, in_=out_t[:].rearrange("p q n -> p (q n)"))
    nc.compile()
    return nc


def _get_nc():
    if "nc" not in _CACHE:
        _CACHE["nc"] = _build_nc()
    return _CACHE["nc"]


def _run(in_maps, trace=False):
    from concourse.bass_utils import run_bass_kernel_spmd
    return run_bass_kernel_spmd(_get_nc(), in_maps, core_ids=list(range(8)),
                                trace=trace)


def _assemble(feats_list):
    out = np.zeros((1, 512, IMH, IMW), np.float32)
    ii = np.arange(3)
    jj = np.arange(6)
    for c in range(8):
        oy, ox, h = (c >> 2) & 1, (c >> 1) & 1, c & 1
        f = (feats_list[c].reshape(128, 4, 72).transpose(1, 0, 2)
             .reshape(512, 4, 3, 6))
        for py in range(2):
            for px in range(2):
                i_idx = 4 * (3 * h + ii) + 2 * py + oy
                j_idx = 4 * jj + 2 * px + ox
                out[0, :, i_idx[:, None], j_idx[None, :]] = (
                    f[:, py * 2 + px].transpose(1, 2, 0)
                )
    return out


def kernel(**inputs):
    r1s, w1, w2, w3, w45d, biases = _host_prep(**inputs)
    in_maps = [
        {"r1": r1s[c], "w1": w1, "w2": w2, "w3": w3, "w45d": w45d, "biases": biases}
        for c in range(8)
    ]
    res = _run(in_maps)
    feats_list = [res.results[c]["feats"] for c in range(8)]
    return _assemble(feats_list)
